# revision 4
# baseline (speedup 1.0000x reference)
"""DiscretizedMixLogisticLoss Bass kernel v2 for TRN2, 8-core data-parallel.

Full inputs: x [8,3,256,256] f32, l [8,120,256,256] f32 -> nll [8,3,256,256] f32.
Sharding: batch dim N=8 across 8 cores (1 example/core).

Math per pixel/channel c/mixture k (l viewed as [4,3,10,HW]):
  s=l[0], mu=l[1], sc=l[2], co=l[3]
  sig = sigmoid(co); m' = clip(mu + coupling(sig*x), 0, 255)
  tlo = (x-0.5-m')*inv, thi = tlo+inv, inv = exp(-sc)
  d = max(sig(thi)-sig(tlo), 1e-12);  nll = ln(sum_k e^s) - ln(sum_k e^s d)

Factored for the engines (doubled vars kill the 0.5s):
  sg=tanh(co/2) [ACT]; tmp2=(sg+1)*cx2 [=2 sig cx]
  Q = -Coup@tmp2 - 2I@mu  (PE, f32)    negmu2 = clip(Q,-510,0)
  z2 = 2x + negmu2 = 2(x-m')
  h = exp(-sc-ln2) = inv/2 [ACT]; tlo=(z2-1)h, thi=(z2+1)h
  sig via exp(-t) -> (min 1e37) +1 -> reciprocal_approx_accurate
  (matches the reference's 1/(1+exp(-t)) f32 rounding; approx error
  cancels between the two nearby branches)
  K-sums ride PE in bf16; Ln batched once at the end (one table switch).
Edge pixels (x<0.001/x>254.999) fixed up on host.
"""
from contextlib import ExitStack

import os

import numpy as np
import ml_dtypes

import concourse.bass as bass
import concourse.bacc as bacc
import concourse.tile as tile
from concourse import mybir
from concourse.bass_utils import run_bass_kernel_spmd

AF = mybir.ActivationFunctionType
ALU = mybir.AluOpType
F32 = mybir.dt.float32
BF16 = mybir.dt.bfloat16

N, C, K, H, W = 8, 3, 10, 256, 256
HW = H * W
CK = C * K            # 30
P = CK * 4            # 120 partitions
NCORES = 8

FP32R = False
GPOFF = False
RFAST = False
J = 1024              # pixels per partition per tile
DEEP = J <= 1024      # smaller tiles -> deeper buffering fits
NT = HW // (4 * J)    # 8 tiles
QS = HW // 4          # 16384, quarter stride

LN_HALF = float(np.log(0.5))


def _l_ap(lt, param, t):
    """DRAM AP over l [120, HW]: partition p=(c*K+k)*4+q -> row param*30+ck,
    pixel q*QS+t*J+j."""
    return bass.AP(tensor=lt, offset=param * CK * HW + t * J,
                   ap=[[HW, CK], [QS, 4], [1, J]])


def _out_ap(out, t):
    return bass.AP(tensor=out, offset=t * J,
                   ap=[[HW, C], [QS, 4], [1, J]])


def build_kernel():
    nc = bacc.Bacc("TRN2", target_bir_lowering=False, debug=False)

    l_in = nc.dram_tensor("l_in", [4 * CK, HW], F32, kind="ExternalInput")
    x_in = nc.dram_tensor("x_in", [C, HW], F32, kind="ExternalInput")
    F32R = mybir.dt.float32r
    wq_dt = F32R if FP32R else F32
    wq_d = nc.dram_tensor("wq", [P, 240], F32R, kind="ExternalInput")
    ws_d = nc.dram_tensor("ws", [P, 48], BF16, kind="ExternalInput")
    out = nc.dram_tensor("out", [C, HW], F32, kind="ExternalOutput")

    with tile.TileContext(nc) as tc, ExitStack() as ctx:
        consts = ctx.enter_context(tc.tile_pool(name="consts", bufs=1))
        nb = 4 if DEEP else 2
        lpool = ctx.enter_context(tc.tile_pool(name="lpool", bufs=nb))
        xpool = ctx.enter_context(tc.tile_pool(name="xpool", bufs=nb))
        wkpool = ctx.enter_context(tc.tile_pool(name="wkpool", bufs=3 if DEEP else 2))
        ttpool = ctx.enter_context(tc.tile_pool(name="ttpool", bufs=nb))
        bpool = ctx.enter_context(tc.tile_pool(name="bpool", bufs=2))
        rpool = ctx.enter_context(tc.tile_pool(name="rpool", bufs=2 if DEEP else 1))
        qpsum = ctx.enter_context(tc.tile_pool(name="qpsum", bufs=2 if DEEP else 1,
                                               space="PSUM"))
        rpsum = ctx.enter_context(tc.tile_pool(name="rpsum", bufs=2 if DEEP else 1,
                                               space="PSUM"))
        stand = ctx.enter_context(tc.tile_pool(name="stand", bufs=1))

        wq = consts.tile([P, 240], F32R)
        ws = consts.tile([P, 48], BF16)
        nc.sync.dma_start(out=wq, in_=wq_d[:, :])
        nc.sync.dma_start(out=ws, in_=ws_d[:, :])
        lnhalf = consts.tile([P, 1], F32)
        nc.vector.memset(lnhalf, LN_HALF)
        w_mu = wq[:, 0:120]                    # -2*I, fp32r (1 cyc/col)
        w_coup = wq[:, 120:240].bitcast(F32)   # -1 coupling (f32: tmp2 is f32)
        w_s1 = ws[:, 0:12]        # den sum -> rows 0:12
        w_s2 = ws[:, 12:24]       # n sum -> rows 32:44

        # standing results: row t*12+(c*4+q); cols [0:J]=r1(den), [J:2J]=r2(n)
        NTA = NT * 5 // 8
        rbufA = stand.tile([NTA * 12, 2 * J], F32, tag="rb")
        rbufB = None

        def _tail(rb, t0, nt):
            nc.scalar.activation(out=rb, in_=rb, func=AF.Ln)
            nc.vector.tensor_tensor(out=rb[:, 0:J], in0=rb[:, 0:J],
                                    in1=rb[:, J:2 * J], op=ALU.subtract)
            for tt_ in range(nt):
                nc.sync.dma_start(out=_out_ap(out, t0 + tt_),
                                  in_=rb[tt_ * 12:(tt_ + 1) * 12, 0:J])

        for t in range(NT):
            lg = lpool.tile([P, J], F32, tag="lg")
            mu = lpool.tile([P, J], F32R, tag="mu")
            sc = lpool.tile([P, J], F32, tag="sc")
            co = lpool.tile([P, J], F32, tag="co")
            # broadcast x from DRAM with stride-0 repeats over k.
            # cxx = [cx2 | cx1]: cx2[cc]=x[chan(cc)] chan=(0,0,1); cx1[c]=x[c]
            cxx = xpool.tile([P, 2 * J], F32, tag="cxx")
            nc.scalar.dma_start(out=co, in_=_l_ap(l_in, 3, t))
            nc.scalar.dma_start(
                out=cxx[0:80, 0:J],
                in_=bass.AP(tensor=x_in, offset=t * J, ap=[[0, 20], [QS, 4], [1, J]]))
            nc.scalar.dma_start(
                out=cxx[80:120, 0:J],
                in_=bass.AP(tensor=x_in, offset=HW + t * J,
                            ap=[[0, 10], [QS, 4], [1, J]]))
            nc.sync.dma_start(out=mu, in_=_l_ap(l_in, 1, t).bitcast(F32R))
            nc.sync.dma_start(out=sc, in_=_l_ap(l_in, 2, t))
            for c in range(C):
                nc.scalar.dma_start(
                    out=cxx[c * 40:(c + 1) * 40, J:2 * J],
                    in_=bass.AP(tensor=x_in, offset=c * HW + t * J,
                                ap=[[0, 10], [QS, 4], [1, J]]))
            nc.sync.dma_start(out=lg, in_=_l_ap(l_in, 0, t))

            # sg = tanh(co/2) in place; tmp2 = (sg+1)*cx2 -> wk right half
            # (rounded to f32r so the coupling matmul can stream at 1 cyc/col)
            wk = wkpool.tile([P, 2 * J], F32, tag="wk")
            tmp2 = wk[:, J:2 * J]
            nc.scalar.activation(out=co, in_=co, func=AF.Tanh, scale=0.5)
            _ev = nc.gpsimd if GPOFF else nc.vector
            nc.vector.scalar_tensor_tensor(
                out=tmp2, in0=co, scalar=1.0, in1=cxx[:, 0:J],
                op0=ALU.add, op1=ALU.mult)

            # Q = -Coup@tmp2 - 2I@mu  (PSUM; fp32r streams 4x faster)
            qp = qpsum.tile([P, J], F32, tag="q")
            for i in range(J // 512):
                s0, s1 = i * 512, (i + 1) * 512
                nc.tensor.matmul(qp[:, s0:s1], w_coup, tmp2[:, s0:s1],
                                 start=True, stop=False)
            for i in range(J // 512):
                s0, s1 = i * 512, (i + 1) * 512
                nc.tensor.matmul(qp[:, s0:s1], w_mu, mu[:, s0:s1],
                                 start=False, stop=True)

            # h = exp(-sc - ln2) = inv/2, in place
            nc.scalar.activation(out=sc, in_=sc, func=AF.Exp, scale=-1.0,
                                 bias=lnhalf[:, :])

            # negmu2 = clip(Q, -510, 0); z2 = 2*cx1 + negmu2 (=2(x-m'))
            nc.vector.tensor_scalar(out=wk[:, 0:J], in0=qp, scalar1=-510.0,
                                    scalar2=0.0, op0=ALU.max, op1=ALU.min)
            nc.vector.scalar_tensor_tensor(
                out=wk[:, 0:J], in0=cxx[:, J:2 * J], scalar=2.0,
                in1=wk[:, 0:J], op0=ALU.mult, op1=ALU.add)

            # tlo = (z2-1)*h ; thi = (z2+1)*h  (packed [tlo|thi])
            tt = ttpool.tile([P, 2 * J], F32, tag="tt")
            nc.vector.scalar_tensor_tensor(
                out=tt[:, 0:J], in0=wk[:, 0:J], scalar=-1.0, in1=sc,
                op0=ALU.add, op1=ALU.mult)
            nc.vector.scalar_tensor_tensor(
                out=tt[:, J:2 * J], in0=wk[:, 0:J], scalar=1.0, in1=sc,
                op0=ALU.add, op1=ALU.mult)

            # sig = 1/(1 + exp(-t)), inf-guarded, approx-recip (~2 ULP).
            # Chunked per half so the ACT exp latency hides behind the other
            # half's DVE work.
            for lo, hi in ((0, J), (J, 2 * J)):
                th = tt[:, lo:hi]
                nc.scalar.activation(out=th, in_=th, func=AF.Exp, scale=-1.0)
                nc.vector.tensor_scalar(out=th, in0=th, scalar1=1e37,
                                        scalar2=1.0, op0=ALU.min, op1=ALU.add)
                if RFAST:
                    nc.vector.reciprocal_approx_fast(out=th, in_=th)
                else:
                    nc.vector.reciprocal_approx_accurate(out=th, in_=th,
                                                         scratch=wk[:, lo:hi])

            # w = exp(lg) -> bf16 ; d = sig_hi - sig_lo ; wd = max(d,1e-12)*w
            wb = bpool.tile([P, J], BF16, tag="wb")
            nc.scalar.activation(out=wb, in_=lg, func=AF.Exp)
            _ev.tensor_tensor(out=wk[:, 0:J], in0=tt[:, J:2 * J],
                              in1=tt[:, 0:J], op=ALU.subtract)
            wdb = bpool.tile([P, J], BF16, tag="wdb")
            nc.vector.scalar_tensor_tensor(
                out=wdb, in0=wk[:, 0:J], scalar=1e-12, in1=wb,
                op0=ALU.max, op1=ALU.mult)

            # K-sums on PE (bf16): rows 0:12 = sum w, rows 32:44 = sum w*d
            rp = rpsum.tile([44, J], F32, tag="rp")
            for i in range(J // 512):
                s0, s1 = i * 512, (i + 1) * 512
                nc.tensor.matmul(rp[0:12, s0:s1], w_s1, wb[:, s0:s1],
                                 start=True, stop=True)
                nc.tensor.matmul(rp[32:44, s0:s1], w_s2, wdb[:, s0:s1],
                                 start=True, stop=True)

            # stage into standing buffer: ACT copy PSUM->scratch (copy is in
            # every act set), then SBUF->SBUF DMA (any partition base ok)
            rsc = rpool.tile([44, J], F32, tag="rsc")
            nc.scalar.copy(out=rsc, in_=rp)
            if t == NTA:
                rbufB = stand.tile([(NT - NTA) * 12, 2 * J], F32, tag="rb")
            rb, tb = (rbufA, t) if t < NTA else (rbufB, t - NTA)
            nc.sync.dma_start(out=rb[tb * 12:(tb + 1) * 12, 0:J],
                              in_=rsc[0:12, :])
            nc.sync.dma_start(out=rb[tb * 12:(tb + 1) * 12, J:2 * J],
                              in_=rsc[32:44, :])
            if t == NTA - 1:
                _tail(rbufA, 0, NTA)

        _tail(rbufB, NTA, NT - NTA)

    nc.compile()
    return nc


_WQ = None
_WS = None
_NC_CACHE = None


def _weights_np():
    global _WQ, _WS
    if _WQ is None:
        wq = np.zeros((P, 240), dtype=np.float32)
        for p in range(P):
            wq[p, p] = -2.0  # -2*I for mu
        cc_to_c = {0: 1, 1: 2, 2: 2}
        for cc in range(C):
            for k in range(K):
                for q in range(4):
                    p = (cc * K + k) * 4 + q
                    col = (cc_to_c[cc] * K + k) * 4 + q
                    wq[p, 120 + col] = -1.0
        ws = np.zeros((P, 48), dtype=np.float32)
        for c in range(C):
            for k in range(K):
                for q in range(4):
                    p = (c * K + k) * 4 + q
                    ws[p, c * 4 + q] = 1.0        # den sum -> rows 0:12
                    ws[p, 12 + c * 4 + q] = 1.0   # n sum -> rows 32:44
        _WQ = wq
        _WS = ws.astype(ml_dtypes.bfloat16)
    return _WQ, _WS


def _host_fixup(nll, x, l):
    """Recompute edge pixels (lo_cond/hi_cond active) exactly on host."""
    f32 = np.float32
    mask = (x < f32(0.001)) | (x > f32(254.999))
    if not mask.any():
        return nll
    l6 = l.reshape(N, 4, C, K, H, W)
    with np.errstate(all="ignore"):
        sg = lambda z: (f32(1) / (f32(1) + np.exp(-z, dtype=f32))).astype(f32)
        for n, cc, hh, ww in zip(*np.nonzero(mask)):
            s = l6[n, 0, cc, :, hh, ww]
            m_raw = l6[n, 1, :, :, hh, ww]
            sc_ = np.maximum(l6[n, 2, cc, :, hh, ww], f32(-7))
            co = sg(l6[n, 3, :, :, hh, ww])
            xpix = x[n, :, hh, ww]
            if cc == 0:
                m = m_raw[0]
            elif cc == 1:
                m = (m_raw[1] + co[0] * xpix[0]).astype(f32)
            else:
                m = (m_raw[2] + co[1] * xpix[0] + co[2] * xpix[1]).astype(f32)
            m = np.clip(m, f32(0), f32(255)).astype(f32)
            cen = (xpix[cc] - m).astype(f32)
            invv = np.exp(-sc_, dtype=f32)
            lo_c = f32(1) if xpix[cc] >= f32(0.001) else f32(0)
            hi_c = f32(1) if xpix[cc] <= f32(254.999) else f32(0)
            cdf_lo = lo_c * sg(invv * (cen - f32(0.5)))
            cdf_hi = hi_c * sg(invv * (cen + f32(0.5))) + (f32(1) - hi_c)
            d = np.maximum(cdf_hi - cdf_lo, f32(1e-12))
            e1 = np.exp(s, dtype=f32)
            e2 = (e1 * d).astype(f32)
            nll[n, cc, hh, ww] = np.log(e1.sum(dtype=f32), dtype=f32) - np.log(
                e2.sum(dtype=f32), dtype=f32)
    return nll


def _get_nc():
    global _NC_CACHE
    if _NC_CACHE is None:
        _NC_CACHE = build_kernel()
    return _NC_CACHE


def _in_maps(x, l):
    wq, ws = _weights_np()
    return [
        {"l_in": l[n].reshape(4 * CK, HW),
         "x_in": x[n].reshape(C, HW),
         "wq": wq, "ws": ws}
        for n in range(NCORES)
    ]


def kernel(x, l):
    x = np.ascontiguousarray(x, dtype=np.float32)
    l = np.ascontiguousarray(l, dtype=np.float32)
    nc = _get_nc()
    res = run_bass_kernel_spmd(nc, _in_maps(x, l), list(range(NCORES))).results
    nll = np.stack([res[n]["out"].reshape(C, H, W) for n in range(NCORES)], axis=0)
    return _host_fixup(nll, x, l)


# revision 5
# speedup vs baseline: 1.0039x; 1.0039x over previous
"""DiscretizedMixLogisticLoss Bass kernel v2 for TRN2, 8-core data-parallel.

Full inputs: x [8,3,256,256] f32, l [8,120,256,256] f32 -> nll [8,3,256,256] f32.
Sharding: batch dim N=8 across 8 cores (1 example/core).

Math per pixel/channel c/mixture k (l viewed as [4,3,10,HW]):
  s=l[0], mu=l[1], sc=l[2], co=l[3]
  sig = sigmoid(co); m' = clip(mu + coupling(sig*x), 0, 255)
  tlo = (x-0.5-m')*inv, thi = tlo+inv, inv = exp(-sc)
  d = max(sig(thi)-sig(tlo), 1e-12);  nll = ln(sum_k e^s) - ln(sum_k e^s d)

Factored for the engines (doubled vars kill the 0.5s):
  sg=tanh(co/2) [ACT]; tmp2=(sg+1)*cx2 [=2 sig cx]
  Q = -Coup@tmp2 - 2I@mu  (PE, f32)    negmu2 = clip(Q,-510,0)
  z2 = 2x + negmu2 = 2(x-m')
  h = exp(-sc-ln2) = inv/2 [ACT]; tlo=(z2-1)h, thi=(z2+1)h
  sig via exp(-t) -> (min 1e37) +1 -> reciprocal_approx_accurate
  (matches the reference's 1/(1+exp(-t)) f32 rounding; approx error
  cancels between the two nearby branches)
  K-sums ride PE in bf16; Ln batched once at the end (one table switch).
Edge pixels (x<0.001/x>254.999) fixed up on host.
"""
from contextlib import ExitStack

import os

import numpy as np
import ml_dtypes

import concourse.bass as bass
import concourse.bacc as bacc
import concourse.tile as tile
from concourse import mybir
from concourse.bass_utils import run_bass_kernel_spmd

AF = mybir.ActivationFunctionType
ALU = mybir.AluOpType
F32 = mybir.dt.float32
BF16 = mybir.dt.bfloat16

N, C, K, H, W = 8, 3, 10, 256, 256
HW = H * W
CK = C * K            # 30
P = CK * 4            # 120 partitions
NCORES = 8

FP32R = False
GPOFF = False
RFAST = False
J = 1024              # pixels per partition per tile
DEEP = J <= 1024      # smaller tiles -> deeper buffering fits
NT = HW // (4 * J)    # 8 tiles
QS = HW // 4          # 16384, quarter stride

LN_HALF = float(np.log(0.5))


def _l_ap(lt, param, t):
    """DRAM AP over l [120, HW]: partition p=(c*K+k)*4+q -> row param*30+ck,
    pixel q*QS+t*J+j."""
    return bass.AP(tensor=lt, offset=param * CK * HW + t * J,
                   ap=[[HW, CK], [QS, 4], [1, J]])


def _out_ap(out, t):
    return bass.AP(tensor=out, offset=t * J,
                   ap=[[HW, C], [QS, 4], [1, J]])


def build_kernel():
    nc = bacc.Bacc("TRN2", target_bir_lowering=False, debug=False)

    l_in = nc.dram_tensor("l_in", [4 * CK, HW], F32, kind="ExternalInput")
    x_in = nc.dram_tensor("x_in", [C, HW], F32, kind="ExternalInput")
    F32R = mybir.dt.float32r
    wq_dt = F32R if FP32R else F32
    wq_d = nc.dram_tensor("wq", [P, 240], F32R, kind="ExternalInput")
    ws_d = nc.dram_tensor("ws", [P, 48], BF16, kind="ExternalInput")
    out = nc.dram_tensor("out", [C, HW], F32, kind="ExternalOutput")

    with tile.TileContext(nc) as tc, ExitStack() as ctx:
        consts = ctx.enter_context(tc.tile_pool(name="consts", bufs=1))
        nb = 3 if DEEP else 2
        lpool = ctx.enter_context(tc.tile_pool(name="lpool", bufs=nb))
        xpool = ctx.enter_context(tc.tile_pool(name="xpool", bufs=nb))
        wkpool = ctx.enter_context(tc.tile_pool(name="wkpool", bufs=2))
        ttpool = ctx.enter_context(tc.tile_pool(name="ttpool", bufs=nb))
        bpool = ctx.enter_context(tc.tile_pool(name="bpool", bufs=2))
        rpool = ctx.enter_context(tc.tile_pool(name="rpool", bufs=2 if DEEP else 1))
        qpsum = ctx.enter_context(tc.tile_pool(name="qpsum", bufs=2 if DEEP else 1,
                                               space="PSUM"))
        rpsum = ctx.enter_context(tc.tile_pool(name="rpsum", bufs=2 if DEEP else 1,
                                               space="PSUM"))
        stand = ctx.enter_context(tc.tile_pool(name="stand", bufs=1))

        wq = consts.tile([P, 240], F32R)
        ws = consts.tile([P, 48], BF16)
        nc.sync.dma_start(out=wq, in_=wq_d[:, :])
        nc.sync.dma_start(out=ws, in_=ws_d[:, :])
        lnhalf = consts.tile([P, 1], F32)
        nc.vector.memset(lnhalf, LN_HALF)
        w_mu = wq[:, 0:120]                    # -2*I, fp32r (1 cyc/col)
        w_coup = wq[:, 120:240].bitcast(F32)   # -1 coupling (f32: tmp2 is f32)
        w_s1 = ws[:, 0:12]        # den sum -> rows 0:12
        w_s2 = ws[:, 12:24]       # n sum -> rows 32:44

        # standing results: row t*12+(c*4+q); cols [0:J]=r1(den), [J:2J]=r2(n)
        NTA = NT * 5 // 8
        rbufA = stand.tile([NTA * 12, 2 * J], F32, tag="rb")
        rbufB = None

        def _tail(rb, t0, nt):
            nc.scalar.activation(out=rb, in_=rb, func=AF.Ln)
            nc.vector.tensor_tensor(out=rb[:, 0:J], in0=rb[:, 0:J],
                                    in1=rb[:, J:2 * J], op=ALU.subtract)
            for tt_ in range(nt):
                nc.sync.dma_start(out=_out_ap(out, t0 + tt_),
                                  in_=rb[tt_ * 12:(tt_ + 1) * 12, 0:J])

        for t in range(NT):
            lg = lpool.tile([P, J], F32, tag="lg")
            mu = lpool.tile([P, J], F32R, tag="mu")
            sc = lpool.tile([P, J], F32, tag="sc")
            co = lpool.tile([P, J], F32, tag="co")
            # broadcast x from DRAM with stride-0 repeats over k.
            # cxx = [cx2 | cx1]: cx2[cc]=x[chan(cc)] chan=(0,0,1); cx1[c]=x[c]
            cxx = xpool.tile([P, 2 * J], F32, tag="cxx")
            nc.scalar.dma_start(out=co, in_=_l_ap(l_in, 3, t))
            nc.scalar.dma_start(
                out=cxx[0:80, 0:J],
                in_=bass.AP(tensor=x_in, offset=t * J, ap=[[0, 20], [QS, 4], [1, J]]))
            nc.scalar.dma_start(
                out=cxx[80:120, 0:J],
                in_=bass.AP(tensor=x_in, offset=HW + t * J,
                            ap=[[0, 10], [QS, 4], [1, J]]))
            nc.sync.dma_start(out=mu, in_=_l_ap(l_in, 1, t).bitcast(F32R))
            nc.sync.dma_start(out=sc, in_=_l_ap(l_in, 2, t))
            for c in range(C):
                nc.scalar.dma_start(
                    out=cxx[c * 40:(c + 1) * 40, J:2 * J],
                    in_=bass.AP(tensor=x_in, offset=c * HW + t * J,
                                ap=[[0, 10], [QS, 4], [1, J]]))
            nc.sync.dma_start(out=lg, in_=_l_ap(l_in, 0, t))

            # sg = tanh(co/2) in place; tmp2 = (sg+1)*cx2 -> wk right half
            # (rounded to f32r so the coupling matmul can stream at 1 cyc/col)
            wk = wkpool.tile([P, 2 * J], F32, tag="wk")
            tmp2 = wk[:, J:2 * J]
            nc.scalar.activation(out=co, in_=co, func=AF.Tanh, scale=0.5)
            _ev = nc.gpsimd if GPOFF else nc.vector
            nc.vector.scalar_tensor_tensor(
                out=tmp2, in0=co, scalar=1.0, in1=cxx[:, 0:J],
                op0=ALU.add, op1=ALU.mult)

            # Q = -Coup@tmp2 - 2I@mu  (PSUM; fp32r streams 4x faster)
            qp = qpsum.tile([P, J], F32, tag="q")
            for i in range(J // 512):
                s0, s1 = i * 512, (i + 1) * 512
                nc.tensor.matmul(qp[:, s0:s1], w_coup, tmp2[:, s0:s1],
                                 start=True, stop=False)
            for i in range(J // 512):
                s0, s1 = i * 512, (i + 1) * 512
                nc.tensor.matmul(qp[:, s0:s1], w_mu, mu[:, s0:s1],
                                 start=False, stop=True)

            # h = exp(-sc - ln2) = inv/2, in place
            nc.scalar.activation(out=sc, in_=sc, func=AF.Exp, scale=-1.0,
                                 bias=lnhalf[:, :])

            # negmu2 = clip(Q, -510, 0); z2 = 2*cx1 + negmu2 (=2(x-m'))
            nc.vector.tensor_scalar(out=wk[:, 0:J], in0=qp, scalar1=-510.0,
                                    scalar2=0.0, op0=ALU.max, op1=ALU.min)
            nc.vector.scalar_tensor_tensor(
                out=wk[:, 0:J], in0=cxx[:, J:2 * J], scalar=2.0,
                in1=wk[:, 0:J], op0=ALU.mult, op1=ALU.add)

            # tlo = (z2-1)*h ; thi = (z2+1)*h  (packed [tlo|thi])
            tt = ttpool.tile([P, 2 * J], F32, tag="tt")
            nc.vector.scalar_tensor_tensor(
                out=tt[:, 0:J], in0=wk[:, 0:J], scalar=-1.0, in1=sc,
                op0=ALU.add, op1=ALU.mult)
            nc.vector.scalar_tensor_tensor(
                out=tt[:, J:2 * J], in0=wk[:, 0:J], scalar=1.0, in1=sc,
                op0=ALU.add, op1=ALU.mult)

            # sig = 1/(1 + exp(-t)), inf-guarded, approx-recip (~2 ULP).
            # Chunked per half so the ACT exp latency hides behind the other
            # half's DVE work.
            for lo, hi in ((0, J), (J, 2 * J)):
                th = tt[:, lo:hi]
                nc.scalar.activation(out=th, in_=th, func=AF.Exp, scale=-1.0)
                nc.vector.tensor_scalar(out=th, in0=th, scalar1=1e37,
                                        scalar2=1.0, op0=ALU.min, op1=ALU.add)
                if RFAST:
                    nc.vector.reciprocal_approx_fast(out=th, in_=th)
                else:
                    nc.vector.reciprocal_approx_accurate(out=th, in_=th,
                                                         scratch=wk[:, lo:hi])

            # w = exp(lg) -> bf16 ; d = sig_hi - sig_lo ; wd = max(d,1e-12)*w
            wb = bpool.tile([P, J], BF16, tag="wb")
            nc.scalar.activation(out=wb, in_=lg, func=AF.Exp)
            _ev.tensor_tensor(out=wk[:, 0:J], in0=tt[:, J:2 * J],
                              in1=tt[:, 0:J], op=ALU.subtract)
            wdb = bpool.tile([P, J], BF16, tag="wdb")
            nc.vector.scalar_tensor_tensor(
                out=wdb, in0=wk[:, 0:J], scalar=1e-12, in1=wb,
                op0=ALU.max, op1=ALU.mult)

            # K-sums on PE (bf16): rows 0:12 = sum w, rows 32:44 = sum w*d
            rp = rpsum.tile([44, J], F32, tag="rp")
            for i in range(J // 512):
                s0, s1 = i * 512, (i + 1) * 512
                nc.tensor.matmul(rp[0:12, s0:s1], w_s1, wb[:, s0:s1],
                                 start=True, stop=True)
                nc.tensor.matmul(rp[32:44, s0:s1], w_s2, wdb[:, s0:s1],
                                 start=True, stop=True)

            # stage into standing buffer: ACT copy PSUM->scratch (copy is in
            # every act set), then SBUF->SBUF DMA (any partition base ok)
            rsc = rpool.tile([44, J], F32, tag="rsc")
            nc.scalar.copy(out=rsc, in_=rp)
            if t == NTA:
                rbufB = stand.tile([(NT - NTA) * 12, 2 * J], F32, tag="rb")
            rb, tb = (rbufA, t) if t < NTA else (rbufB, t - NTA)
            nc.sync.dma_start(out=rb[tb * 12:(tb + 1) * 12, 0:J],
                              in_=rsc[0:12, :])
            nc.sync.dma_start(out=rb[tb * 12:(tb + 1) * 12, J:2 * J],
                              in_=rsc[32:44, :])
            if t == NTA - 1:
                _tail(rbufA, 0, NTA)

        _tail(rbufB, NTA, NT - NTA)

    nc.compile()
    return nc


_WQ = None
_WS = None
_NC_CACHE = None


def _weights_np():
    global _WQ, _WS
    if _WQ is None:
        wq = np.zeros((P, 240), dtype=np.float32)
        for p in range(P):
            wq[p, p] = -2.0  # -2*I for mu
        cc_to_c = {0: 1, 1: 2, 2: 2}
        for cc in range(C):
            for k in range(K):
                for q in range(4):
                    p = (cc * K + k) * 4 + q
                    col = (cc_to_c[cc] * K + k) * 4 + q
                    wq[p, 120 + col] = -1.0
        ws = np.zeros((P, 48), dtype=np.float32)
        for c in range(C):
            for k in range(K):
                for q in range(4):
                    p = (c * K + k) * 4 + q
                    ws[p, c * 4 + q] = 1.0        # den sum -> rows 0:12
                    ws[p, 12 + c * 4 + q] = 1.0   # n sum -> rows 32:44
        _WQ = wq
        _WS = ws.astype(ml_dtypes.bfloat16)
    return _WQ, _WS


def _host_fixup(nll, x, l):
    """Recompute edge pixels (lo_cond/hi_cond active) exactly on host."""
    f32 = np.float32
    mask = (x < f32(0.001)) | (x > f32(254.999))
    if not mask.any():
        return nll
    l6 = l.reshape(N, 4, C, K, H, W)
    with np.errstate(all="ignore"):
        sg = lambda z: (f32(1) / (f32(1) + np.exp(-z, dtype=f32))).astype(f32)
        for n, cc, hh, ww in zip(*np.nonzero(mask)):
            s = l6[n, 0, cc, :, hh, ww]
            m_raw = l6[n, 1, :, :, hh, ww]
            sc_ = np.maximum(l6[n, 2, cc, :, hh, ww], f32(-7))
            co = sg(l6[n, 3, :, :, hh, ww])
            xpix = x[n, :, hh, ww]
            if cc == 0:
                m = m_raw[0]
            elif cc == 1:
                m = (m_raw[1] + co[0] * xpix[0]).astype(f32)
            else:
                m = (m_raw[2] + co[1] * xpix[0] + co[2] * xpix[1]).astype(f32)
            m = np.clip(m, f32(0), f32(255)).astype(f32)
            cen = (xpix[cc] - m).astype(f32)
            invv = np.exp(-sc_, dtype=f32)
            lo_c = f32(1) if xpix[cc] >= f32(0.001) else f32(0)
            hi_c = f32(1) if xpix[cc] <= f32(254.999) else f32(0)
            cdf_lo = lo_c * sg(invv * (cen - f32(0.5)))
            cdf_hi = hi_c * sg(invv * (cen + f32(0.5))) + (f32(1) - hi_c)
            d = np.maximum(cdf_hi - cdf_lo, f32(1e-12))
            e1 = np.exp(s, dtype=f32)
            e2 = (e1 * d).astype(f32)
            nll[n, cc, hh, ww] = np.log(e1.sum(dtype=f32), dtype=f32) - np.log(
                e2.sum(dtype=f32), dtype=f32)
    return nll


def _get_nc():
    global _NC_CACHE
    if _NC_CACHE is None:
        _NC_CACHE = build_kernel()
    return _NC_CACHE


def _in_maps(x, l):
    wq, ws = _weights_np()
    return [
        {"l_in": l[n].reshape(4 * CK, HW),
         "x_in": x[n].reshape(C, HW),
         "wq": wq, "ws": ws}
        for n in range(NCORES)
    ]


def kernel(x, l):
    x = np.ascontiguousarray(x, dtype=np.float32)
    l = np.ascontiguousarray(l, dtype=np.float32)
    nc = _get_nc()
    res = run_bass_kernel_spmd(nc, _in_maps(x, l), list(range(NCORES))).results
    nll = np.stack([res[n]["out"].reshape(C, H, W) for n in range(NCORES)], axis=0)
    return _host_fixup(nll, x, l)


# revision 6
# speedup vs baseline: 1.0248x; 1.0208x over previous
"""DiscretizedMixLogisticLoss Bass kernel v2 for TRN2, 8-core data-parallel.

Full inputs: x [8,3,256,256] f32, l [8,120,256,256] f32 -> nll [8,3,256,256] f32.
Sharding: batch dim N=8 across 8 cores (1 example/core).

Math per pixel/channel c/mixture k (l viewed as [4,3,10,HW]):
  s=l[0], mu=l[1], sc=l[2], co=l[3]
  sig = sigmoid(co); m' = clip(mu + coupling(sig*x), 0, 255)
  tlo = (x-0.5-m')*inv, thi = tlo+inv, inv = exp(-sc)
  d = max(sig(thi)-sig(tlo), 1e-12);  nll = ln(sum_k e^s) - ln(sum_k e^s d)

Factored for the engines (doubled vars kill the 0.5s):
  sg=tanh(co/2) [ACT]; tmp2=(sg+1)*cx2 [=2 sig cx]
  Q = -Coup@tmp2 - 2I@mu  (PE, f32)    negmu2 = clip(Q,-510,0)
  z2 = 2x + negmu2 = 2(x-m')
  h = exp(-sc-ln2) = inv/2 [ACT]; tlo=(z2-1)h, thi=(z2+1)h
  sig via exp(-t) -> (min 1e37) +1 -> reciprocal_approx_accurate
  (matches the reference's 1/(1+exp(-t)) f32 rounding; approx error
  cancels between the two nearby branches)
  K-sums ride PE in bf16; Ln batched once at the end (one table switch).
Edge pixels (x<0.001/x>254.999) fixed up on host.
"""
from contextlib import ExitStack

import os

import numpy as np
import ml_dtypes

import concourse.bass as bass
import concourse.bacc as bacc
import concourse.tile as tile
from concourse import mybir
from concourse.bass_utils import run_bass_kernel_spmd

AF = mybir.ActivationFunctionType
ALU = mybir.AluOpType
F32 = mybir.dt.float32
BF16 = mybir.dt.bfloat16

N, C, K, H, W = 8, 3, 10, 256, 256
HW = H * W
CK = C * K            # 30
P = CK * 4            # 120 partitions
NCORES = 8

FP32R = False
GPOFF = False
RFAST = False
J = 1024              # pixels per partition per tile
DEEP = J <= 1024      # smaller tiles -> deeper buffering fits
NT = HW // (4 * J)    # 8 tiles
QS = HW // 4          # 16384, quarter stride

LN_HALF = float(np.log(0.5))


def _l_ap(lt, param, t):
    """DRAM AP over l [120, HW]: partition p=(c*K+k)*4+q -> row param*30+ck,
    pixel q*QS+t*J+j."""
    return bass.AP(tensor=lt, offset=param * CK * HW + t * J,
                   ap=[[HW, CK], [QS, 4], [1, J]])


def _out_ap(out, t):
    return bass.AP(tensor=out, offset=t * J,
                   ap=[[HW, C], [QS, 4], [1, J]])


def build_kernel():
    nc = bacc.Bacc("TRN2", target_bir_lowering=False, debug=False)

    l_in = nc.dram_tensor("l_in", [4 * CK, HW], F32, kind="ExternalInput")
    x_in = nc.dram_tensor("x_in", [C, HW], F32, kind="ExternalInput")
    F32R = mybir.dt.float32r
    wq_dt = F32R if FP32R else F32
    wq_d = nc.dram_tensor("wq", [P, 240], F32R, kind="ExternalInput")
    ws_d = nc.dram_tensor("ws", [P, 48], BF16, kind="ExternalInput")
    out = nc.dram_tensor("out", [C, HW], F32, kind="ExternalOutput")

    with tile.TileContext(nc) as tc, ExitStack() as ctx:
        consts = ctx.enter_context(tc.tile_pool(name="consts", bufs=1))
        nb = 3 if DEEP else 2
        lpool = ctx.enter_context(tc.tile_pool(name="lpool", bufs=nb))
        xpool = ctx.enter_context(tc.tile_pool(name="xpool", bufs=nb))
        wkpool = ctx.enter_context(tc.tile_pool(name="wkpool", bufs=2))
        ttpool = ctx.enter_context(tc.tile_pool(name="ttpool", bufs=nb))
        bpool = ctx.enter_context(tc.tile_pool(name="bpool", bufs=2))
        rpool = ctx.enter_context(tc.tile_pool(name="rpool", bufs=2 if DEEP else 1))
        qpsum = ctx.enter_context(tc.tile_pool(name="qpsum", bufs=2 if DEEP else 1,
                                               space="PSUM"))
        rpsum = ctx.enter_context(tc.tile_pool(name="rpsum", bufs=2 if DEEP else 1,
                                               space="PSUM"))
        stand = ctx.enter_context(tc.tile_pool(name="stand", bufs=1))

        wq = consts.tile([P, 240], F32R)
        ws = consts.tile([P, 48], BF16)
        nc.sync.dma_start(out=wq, in_=wq_d[:, :])
        nc.sync.dma_start(out=ws, in_=ws_d[:, :])
        lnhalf = consts.tile([P, 1], F32)
        nc.vector.memset(lnhalf, LN_HALF)
        w_mu = wq[:, 0:120]                    # -2*I, fp32r (1 cyc/col)
        w_coup = wq[:, 120:240].bitcast(F32)   # -1 coupling (f32: tmp2 is f32)
        w_s1 = ws[:, 0:12]        # den sum -> rows 0:12
        w_s2 = ws[:, 12:24]       # n sum -> rows 32:44

        # standing results: row t*12+(c*4+q); cols [0:J]=r1(den), [J:2J]=r2(n)
        NTA = NT * 5 // 8
        rbufA = stand.tile([NTA * 12, 2 * J], F32, tag="rb")
        rbufB = None

        def _tail(rb, t0, nt):
            nc.scalar.activation(out=rb, in_=rb, func=AF.Ln)
            nc.vector.tensor_tensor(out=rb[:, 0:J], in0=rb[:, 0:J],
                                    in1=rb[:, J:2 * J], op=ALU.subtract)
            for tt_ in range(nt):
                nc.sync.dma_start(out=_out_ap(out, t0 + tt_),
                                  in_=rb[tt_ * 12:(tt_ + 1) * 12, 0:J])

        for t in range(NT):
            lg = lpool.tile([P, J], F32, tag="lg")
            mu = lpool.tile([P, J], F32R, tag="mu")
            sc = lpool.tile([P, J], F32, tag="sc")
            co = lpool.tile([P, J], F32, tag="co")
            # broadcast x from DRAM with stride-0 repeats over k.
            # cxx = [cx2 | cx1]: cx2[cc]=x[chan(cc)] chan=(0,0,1); cx1[c]=x[c]
            cxx = xpool.tile([P, 2 * J], F32, tag="cxx")
            nc.gpsimd.dma_start(out=co, in_=_l_ap(l_in, 3, t))
            nc.gpsimd.dma_start(
                out=cxx[0:80, 0:J],
                in_=bass.AP(tensor=x_in, offset=t * J, ap=[[0, 20], [QS, 4], [1, J]]))
            nc.gpsimd.dma_start(
                out=cxx[80:120, 0:J],
                in_=bass.AP(tensor=x_in, offset=HW + t * J,
                            ap=[[0, 10], [QS, 4], [1, J]]))
            nc.sync.dma_start(out=mu, in_=_l_ap(l_in, 1, t).bitcast(F32R))
            nc.sync.dma_start(out=sc, in_=_l_ap(l_in, 2, t))
            for c in range(C):
                nc.gpsimd.dma_start(
                    out=cxx[c * 40:(c + 1) * 40, J:2 * J],
                    in_=bass.AP(tensor=x_in, offset=c * HW + t * J,
                                ap=[[0, 10], [QS, 4], [1, J]]))
            nc.sync.dma_start(out=lg, in_=_l_ap(l_in, 0, t))

            # sg = tanh(co/2) in place; tmp2 = (sg+1)*cx2 -> wk right half
            # (rounded to f32r so the coupling matmul can stream at 1 cyc/col)
            wk = wkpool.tile([P, 2 * J], F32, tag="wk")
            tmp2 = wk[:, J:2 * J]
            nc.scalar.activation(out=co, in_=co, func=AF.Tanh, scale=0.5)
            _ev = nc.gpsimd if GPOFF else nc.vector
            nc.vector.scalar_tensor_tensor(
                out=tmp2, in0=co, scalar=1.0, in1=cxx[:, 0:J],
                op0=ALU.add, op1=ALU.mult)

            # Q = -Coup@tmp2 - 2I@mu  (PSUM; fp32r streams 4x faster)
            qp = qpsum.tile([P, J], F32, tag="q")
            for i in range(J // 512):
                s0, s1 = i * 512, (i + 1) * 512
                nc.tensor.matmul(qp[:, s0:s1], w_coup, tmp2[:, s0:s1],
                                 start=True, stop=False)
            for i in range(J // 512):
                s0, s1 = i * 512, (i + 1) * 512
                nc.tensor.matmul(qp[:, s0:s1], w_mu, mu[:, s0:s1],
                                 start=False, stop=True)

            # h = exp(-sc - ln2) = inv/2, in place
            nc.scalar.activation(out=sc, in_=sc, func=AF.Exp, scale=-1.0,
                                 bias=lnhalf[:, :])

            # negmu2 = clip(Q, -510, 0); z2 = 2*cx1 + negmu2 (=2(x-m'))
            nc.vector.tensor_scalar(out=wk[:, 0:J], in0=qp, scalar1=-510.0,
                                    scalar2=0.0, op0=ALU.max, op1=ALU.min)
            nc.vector.scalar_tensor_tensor(
                out=wk[:, 0:J], in0=cxx[:, J:2 * J], scalar=2.0,
                in1=wk[:, 0:J], op0=ALU.mult, op1=ALU.add)

            # tlo = (z2-1)*h ; thi = (z2+1)*h  (packed [tlo|thi])
            tt = ttpool.tile([P, 2 * J], F32, tag="tt")
            nc.vector.scalar_tensor_tensor(
                out=tt[:, 0:J], in0=wk[:, 0:J], scalar=-1.0, in1=sc,
                op0=ALU.add, op1=ALU.mult)
            nc.vector.scalar_tensor_tensor(
                out=tt[:, J:2 * J], in0=wk[:, 0:J], scalar=1.0, in1=sc,
                op0=ALU.add, op1=ALU.mult)

            # sig = 1/(1 + exp(-t)), inf-guarded, approx-recip (~2 ULP).
            # Chunked per half so the ACT exp latency hides behind the other
            # half's DVE work.
            for lo, hi in ((0, J), (J, 2 * J)):
                th = tt[:, lo:hi]
                nc.scalar.activation(out=th, in_=th, func=AF.Exp, scale=-1.0)
                nc.vector.tensor_scalar(out=th, in0=th, scalar1=1e37,
                                        scalar2=1.0, op0=ALU.min, op1=ALU.add)
                if RFAST:
                    nc.vector.reciprocal_approx_fast(out=th, in_=th)
                else:
                    nc.vector.reciprocal_approx_accurate(out=th, in_=th,
                                                         scratch=wk[:, lo:hi])

            # w = exp(lg) -> bf16 ; d = sig_hi - sig_lo ; wd = max(d,1e-12)*w
            wb = bpool.tile([P, J], BF16, tag="wb")
            nc.scalar.activation(out=wb, in_=lg, func=AF.Exp)
            _ev.tensor_tensor(out=wk[:, 0:J], in0=tt[:, J:2 * J],
                              in1=tt[:, 0:J], op=ALU.subtract)
            wdb = bpool.tile([P, J], BF16, tag="wdb")
            nc.vector.scalar_tensor_tensor(
                out=wdb, in0=wk[:, 0:J], scalar=1e-12, in1=wb,
                op0=ALU.max, op1=ALU.mult)

            # K-sums on PE (bf16): rows 0:12 = sum w, rows 32:44 = sum w*d
            rp = rpsum.tile([44, J], F32, tag="rp")
            for i in range(J // 512):
                s0, s1 = i * 512, (i + 1) * 512
                nc.tensor.matmul(rp[0:12, s0:s1], w_s1, wb[:, s0:s1],
                                 start=True, stop=True)
                nc.tensor.matmul(rp[32:44, s0:s1], w_s2, wdb[:, s0:s1],
                                 start=True, stop=True)

            # stage into standing buffer: ACT copy PSUM->scratch (copy is in
            # every act set), then SBUF->SBUF DMA (any partition base ok)
            rsc = rpool.tile([44, J], F32, tag="rsc")
            nc.scalar.copy(out=rsc, in_=rp)
            if t == NTA:
                rbufB = stand.tile([(NT - NTA) * 12, 2 * J], F32, tag="rb")
            rb, tb = (rbufA, t) if t < NTA else (rbufB, t - NTA)
            nc.sync.dma_start(out=rb[tb * 12:(tb + 1) * 12, 0:J],
                              in_=rsc[0:12, :])
            nc.sync.dma_start(out=rb[tb * 12:(tb + 1) * 12, J:2 * J],
                              in_=rsc[32:44, :])
            if t == NTA - 1:
                _tail(rbufA, 0, NTA)

        _tail(rbufB, NTA, NT - NTA)

    nc.compile()
    return nc


_WQ = None
_WS = None
_NC_CACHE = None


def _weights_np():
    global _WQ, _WS
    if _WQ is None:
        wq = np.zeros((P, 240), dtype=np.float32)
        for p in range(P):
            wq[p, p] = -2.0  # -2*I for mu
        cc_to_c = {0: 1, 1: 2, 2: 2}
        for cc in range(C):
            for k in range(K):
                for q in range(4):
                    p = (cc * K + k) * 4 + q
                    col = (cc_to_c[cc] * K + k) * 4 + q
                    wq[p, 120 + col] = -1.0
        ws = np.zeros((P, 48), dtype=np.float32)
        for c in range(C):
            for k in range(K):
                for q in range(4):
                    p = (c * K + k) * 4 + q
                    ws[p, c * 4 + q] = 1.0        # den sum -> rows 0:12
                    ws[p, 12 + c * 4 + q] = 1.0   # n sum -> rows 32:44
        _WQ = wq
        _WS = ws.astype(ml_dtypes.bfloat16)
    return _WQ, _WS


def _host_fixup(nll, x, l):
    """Recompute edge pixels (lo_cond/hi_cond active) exactly on host."""
    f32 = np.float32
    mask = (x < f32(0.001)) | (x > f32(254.999))
    if not mask.any():
        return nll
    l6 = l.reshape(N, 4, C, K, H, W)
    with np.errstate(all="ignore"):
        sg = lambda z: (f32(1) / (f32(1) + np.exp(-z, dtype=f32))).astype(f32)
        for n, cc, hh, ww in zip(*np.nonzero(mask)):
            s = l6[n, 0, cc, :, hh, ww]
            m_raw = l6[n, 1, :, :, hh, ww]
            sc_ = np.maximum(l6[n, 2, cc, :, hh, ww], f32(-7))
            co = sg(l6[n, 3, :, :, hh, ww])
            xpix = x[n, :, hh, ww]
            if cc == 0:
                m = m_raw[0]
            elif cc == 1:
                m = (m_raw[1] + co[0] * xpix[0]).astype(f32)
            else:
                m = (m_raw[2] + co[1] * xpix[0] + co[2] * xpix[1]).astype(f32)
            m = np.clip(m, f32(0), f32(255)).astype(f32)
            cen = (xpix[cc] - m).astype(f32)
            invv = np.exp(-sc_, dtype=f32)
            lo_c = f32(1) if xpix[cc] >= f32(0.001) else f32(0)
            hi_c = f32(1) if xpix[cc] <= f32(254.999) else f32(0)
            cdf_lo = lo_c * sg(invv * (cen - f32(0.5)))
            cdf_hi = hi_c * sg(invv * (cen + f32(0.5))) + (f32(1) - hi_c)
            d = np.maximum(cdf_hi - cdf_lo, f32(1e-12))
            e1 = np.exp(s, dtype=f32)
            e2 = (e1 * d).astype(f32)
            nll[n, cc, hh, ww] = np.log(e1.sum(dtype=f32), dtype=f32) - np.log(
                e2.sum(dtype=f32), dtype=f32)
    return nll


def _get_nc():
    global _NC_CACHE
    if _NC_CACHE is None:
        _NC_CACHE = build_kernel()
    return _NC_CACHE


def _in_maps(x, l):
    wq, ws = _weights_np()
    return [
        {"l_in": l[n].reshape(4 * CK, HW),
         "x_in": x[n].reshape(C, HW),
         "wq": wq, "ws": ws}
        for n in range(NCORES)
    ]


def kernel(x, l):
    x = np.ascontiguousarray(x, dtype=np.float32)
    l = np.ascontiguousarray(l, dtype=np.float32)
    nc = _get_nc()
    res = run_bass_kernel_spmd(nc, _in_maps(x, l), list(range(NCORES))).results
    nll = np.stack([res[n]["out"].reshape(C, H, W) for n in range(NCORES)], axis=0)
    return _host_fixup(nll, x, l)
